# revision 57
# baseline (speedup 1.0000x reference)
"""Trainium2 Bass kernel for nn_Decoder (worker/task label-probability decoder).

Math:
    worker_feature = inputs[:2048, :64]          # [Wn, A]
    tau            = inputs[2048:, :16]          # [T, L]
    p1 = sigmoid(worker_feature @ W + b)         # [Wn, 1]
    p2 = (1 - p1) / (L - 1)
    P[i, j, l] = p1[i]^tau[j,l] * p2[i]^(1 - tau[j,l])
               = exp(a[i] * tau[j,l] + c[i]),  a = ln p1 - ln p2, c = ln p2

Sharding: pure data parallel over the worker axis (dim 0), 256 workers per
core across 8 cores; tau/W/b replicated. No communication.

Per-core schedule: workers live on SBUF partitions (2 groups of 128), the
flattened task axis streams through PSUM in 2048-column tiles. tau arrives
as an exact 3-term bf16 split laid out [80, 2048] (hi/mid/lo stripe blocks
at partitions 0/32/64, one 2048-wide stripe per partition row) so the whole
thing loads in a single ~1.6us DMA. The tensor engine replicates each
stripe to all 128 partitions with one selector matmul per 512 columns
(sel3 picks the stripe's hi+mid+lo rows, summing the split exactly). The
Exp(a*tau + c) activations run on ACT with per-partition scale/bias - ACT
is the critical resource at ~0.9ns/col - while the output streams to HBM
in 2048-column writes round-robined over the SP and Pool DMA queues so
neither queue exceeds the ACT budget.
"""

import numpy as np

try:
    import concourse.bass as bass  # noqa: F401
except ImportError:  # fall back to the container's repo checkout
    import sys

    for _p in ("/root/.axon_site/_ro/trn_rl_repo", "/opt/trn_rl_repo"):
        if _p not in sys.path:
            sys.path.append(_p)

import concourse.bass as bass
import concourse.tile as tile
from concourse import mybir
from concourse.bass_utils import run_bass_kernel_spmd

WN = 2048  # workers total
TN = 2048  # tasks
L = 16  # edge types / labels
A = 64  # ability features
AA = A + 1  # features + bias column folded in
NCORES = 8
WPC = WN // NCORES  # workers per core (256)
G = WPC // 128  # partition groups per core (2)
F = TN * L  # flattened task axis (32768)

NST = 16  # tau stripes
STW = F // NST  # stripe width (2048)
MM = 512  # matmul columns per instruction (one PSUM bank)
PSW = 2048  # psum tile width (4 banks)
SPL = 80  # tau3/sel3 partition extent (hi@0, mid@32, lo@64)

_AF = mybir.ActivationFunctionType
_f32 = mybir.dt.float32
_bf16 = mybir.dt.bfloat16

WRITE_ENGINES = ("sync", "gpsimd")


class _TC(tile.TileContext):
    """TileContext legalized for a walrus that allows one sync-wait per inst.

    After Tile's normal scheduling + the exit drain/barrier, rewrite every
    multi-wait instruction into a chain of same-engine NOPs (one wait each)
    followed by the instruction with the final wait.
    """

    def _drain_and_barrier(self, tick_clock, wait_clock):
        super()._drain_and_barrier(tick_clock, wait_clock)
        self._split_multi_waits()

    def _fresh_nop(self, engine):
        inst = self.nc.engines[engine].nop(nofuse=True).ins
        self.nc.cur_bb.bb.instructions.remove(inst)
        return inst

    def _split_multi_waits(self):
        for fn in self.nc.m.functions:
            for bb in fn.blocks:
                snapshot = list(bb.instructions)
                if not any(
                    inst.sync_info and len(inst.sync_info.on_wait) > 1
                    for inst in snapshot
                ):
                    continue
                new = []
                for inst in snapshot:
                    si = inst.sync_info
                    if si is not None and si.on_wait and len(si.on_wait) > 1:
                        waits = list(si.on_wait)
                        si.on_wait = waits[-1:]
                        inst.sync_info = si
                        for wt in waits[:-1]:
                            nop = self._fresh_nop(inst.engine)
                            nop.sync_info = mybir.SyncInfo(on_wait=[wt], on_update=[])
                            new.append(nop)
                    new.append(inst)
                bb.instructions[:] = new


def build_nc():
    nc = bass.Bass("TRN2")
    wf = nc.dram_tensor("wf", [WPC, AA], _f32, kind="ExternalInput")
    tau3_in = nc.dram_tensor("tau3", [SPL, STW], _bf16, kind="ExternalInput")
    sel3_in = nc.dram_tensor("sel3", [SPL, NST * 128], _bf16, kind="ExternalInput")
    w_in = nc.dram_tensor("W", [AA], _f32, kind="ExternalInput")
    out = nc.dram_tensor("out", [G, 128, F], _f32, kind="ExternalOutput")

    with _TC(nc) as tc:
        with (
            tc.tile_pool(name="const", bufs=1) as const,
            tc.tile_pool(name="outs", bufs=4) as outs,
            tc.tile_pool(name="psum", bufs=2, space="PSUM") as psum,
        ):
            # ---- activation-table priming (Exp/Ln share one func set);
            # runs while the DMAs below are in flight ----
            zeros = const.tile([128, 1], _f32)
            nc.vector.memset(zeros, 0.0)
            prime = const.tile([128, 1], _f32)
            nc.scalar.activation(prime, zeros, _AF.Exp)

            # ---- leading loads: worker features (bias folded in as
            # feature column A) on SP, augmented weights broadcast on Pool --
            wf_sb = const.tile([128, G, AA], _f32)
            nc.sync.dma_start(
                out=wf_sb, in_=wf[:].rearrange("(g p) a -> p g a", p=128)
            )
            w_ap = w_in[:]
            w_sb = const.tile([128, AA], _f32)
            nc.gpsimd.dma_start(
                out=w_sb,
                in_=bass.AP(tensor=w_ap.tensor, offset=w_ap.offset, ap=[[0, 128], [1, AA]]),
            )

            # ---- tau 3-term bf16 split [80, 2048] + stripe selectors,
            # column-sliced over the DMA queues so the first stripe's
            # operands land as early as possible ----
            tau3 = const.tile([SPL, STW], _bf16)
            sel3 = const.tile([SPL, NST * 128], _bf16)
            LOADQ = ("sync", "gpsimd", "scalar", "sync")
            for k in range(4):
                eng = getattr(nc, LOADQ[k])
                eng.dma_start(
                    out=sel3[:, k * 4 * 128 : (k + 1) * 4 * 128],
                    in_=sel3_in[:, k * 4 * 128 : (k + 1) * 4 * 128],
                )
                eng.dma_start(
                    out=tau3[:, k * MM : (k + 1) * MM],
                    in_=tau3_in[:, k * MM : (k + 1) * MM],
                )

            # ---- per-worker scalars: a = ln p1 - ln p2, c = ln p2 ----
            x = const.tile([128, G], _f32)
            for g in range(G):
                prod = const.tile([128, AA], _f32, tag=f"prod{g}")
                nc.vector.tensor_mul(prod, wf_sb[:, g, :], w_sb)
                nc.vector.reduce_sum(x[:, g : g + 1], prod, axis=mybir.AxisListType.X)

            # e = exp(-(x + b));  p1 = 1 / (1 + e);  p2 = (1 - p1) / 15
            e = const.tile([128, G], _f32)
            nc.scalar.activation(e, x, _AF.Exp, bias=0.0, scale=-1.0)
            nc.vector.tensor_scalar_add(e, e, 1.0)
            pack = const.tile([128, 2 * G], _f32)
            nc.vector.reciprocal(pack[:, 0:G], e)
            nc.vector.tensor_scalar(
                pack[:, G : 2 * G],
                pack[:, 0:G],
                scalar1=-1.0 / (L - 1),
                scalar2=1.0 / (L - 1),
                op0=mybir.AluOpType.mult,
                op1=mybir.AluOpType.add,
            )
            lp = const.tile([128, 2 * G], _f32)
            for g in range(G):
                nc.scalar.activation(
                    lp[:, g :: G], pack[:, g :: G], _AF.Ln
                )
            lp2 = lp[:, G : 2 * G]
            a_sb = const.tile([128, G], _f32)
            for g in range(G):
                nc.vector.tensor_sub(
                    a_sb[:, g : g + 1], lp[:, g : g + 1], lp[:, G + g : G + g + 1]
                )

            # ---- main loop: selector-matmul bcast -> Exp -> stream out ----
            wr = 1
            for s in range(NST):  # one 2048-col stripe per iteration
                c0 = s * STW
                pt = psum.tile([128, PSW], _f32, tag="pt", name=f"pt{s}")
                for n in reversed(range(PSW // MM)):
                    nc.tensor.matmul(
                        pt[:, n * MM : (n + 1) * MM],
                        sel3[:, s * 128 : (s + 1) * 128],
                        tau3[:, n * MM : (n + 1) * MM],
                        start=True,
                        stop=True,
                    )
                for gi, g in enumerate((0, 1) if s % 2 == 0 else (1, 0)):
                    ot = outs.tile([128, PSW], _f32, tag=f"ot{g}", name=f"ot{g}_{s}")
                    nc.scalar.activation(
                        ot,
                        pt,
                        _AF.Exp,
                        bias=lp2[:, g : g + 1],
                        scale=a_sb[:, g : g + 1],
                    )
                    if s == NST - 1:
                        # split the final writes so the drain only waits on
                        # a quarter-stripe's DMA latency
                        finq = (
                            ("sync", "gpsimd", "sync", "gpsimd")
                            if gi == 0
                            else ("scalar", "sync", "gpsimd", "scalar")
                        )
                        for q in range(4):
                            getattr(nc, finq[q]).dma_start(
                                out=out[g, :, c0 + q * MM : c0 + (q + 1) * MM],
                                in_=ot[:, q * MM : (q + 1) * MM],
                            )
                        wr += 1
                    else:
                        getattr(nc, WRITE_ENGINES[wr % len(WRITE_ENGINES)]).dma_start(
                            out=out[g, :, c0 : c0 + PSW], in_=ot
                        )
                        wr += 1
    return nc


def _tau3_split(tau):
    """Exact 3-term bf16 split of tau [F] -> [80, 2048] (hi@0, mid@32, lo@64)."""
    import ml_dtypes

    bf = ml_dtypes.bfloat16
    hi = tau.astype(bf)
    r1 = tau - hi.astype(np.float32)
    mid = r1.astype(bf)
    lo = (r1 - mid.astype(np.float32)).astype(bf)
    out = np.zeros((SPL, STW), dtype=bf)
    out[0:NST] = hi.reshape(NST, STW)
    out[32 : 32 + NST] = mid.reshape(NST, STW)
    out[64 : 64 + NST] = lo.reshape(NST, STW)
    return out


def _selector3():
    """sel3[k, s*128 + p] = 1 for k in {s, 32+s, 64+s} (sums the 3-term split)."""
    import ml_dtypes

    sel = np.zeros((SPL, NST * 128), dtype=ml_dtypes.bfloat16)
    for s in range(NST):
        for base in (0, 32, 64):
            sel[base + s, s * 128 : (s + 1) * 128] = 1.0
    return sel


_NC = None


def kernel(inputs, W, b, worker_num=WN, task_num=TN, edge_type=L, ability_num=A, **_kw):
    global _NC
    inputs = np.ascontiguousarray(np.asarray(inputs, dtype=np.float32))
    W = np.asarray(W, dtype=np.float32).reshape(A)
    b = np.asarray(b, dtype=np.float32).reshape(1)
    assert inputs.shape == (WN + TN, A)

    wf = np.concatenate(
        [inputs[:WN, :A], np.ones((WN, 1), dtype=np.float32)], axis=1
    )
    W_aug = np.concatenate([W, b]).astype(np.float32)
    tau = np.ascontiguousarray(inputs[WN:, :L].reshape(F))
    tau3 = _tau3_split(tau)
    sel3 = _selector3()

    if _NC is None:
        _NC = build_nc()

    in_maps = [
        {
            "wf": np.ascontiguousarray(wf[k * WPC : (k + 1) * WPC]),
            "tau3": tau3,
            "sel3": sel3,
            "W": W_aug,
        }
        for k in range(NCORES)
    ]
    res = run_bass_kernel_spmd(_NC, in_maps, core_ids=list(range(NCORES)))
    parts = [r["out"].reshape(WPC, TN, L) for r in res.results]
    return np.concatenate(parts, axis=0)
